# revision 4
# baseline (speedup 1.0000x reference)
"""Trainium2 Bass kernel for nn_Attention_7945689497706.

v2: pair-processed attention with K=64 row-tiled sim matmuls (PE row
tiling runs both heads' 64-contraction sims concurrently instead of
zero-padding to 128), sim tile [128, 1024] = [h0 i-half | h1 i-half]
so exp stays at full [128,1024] ACT size.
"""

import numpy as np

import concourse.bass as bass
import concourse.mybir as mybir
import concourse.tile as tile
from concourse import bacc
from concourse.bass_utils import run_bass_kernel_spmd

F32 = mybir.dt.float32
F32R = mybir.dt.float32r
BF16 = mybir.dt.bfloat16
AF = mybir.ActivationFunctionType

NCORES = 8
B = 16
C = 512
N = 1024          # pixels = 32*32
HEADS = 8
DH = 64
NMEM = 4
PB = B // NCORES  # batch elements per core
CT = C // 128     # channel partition-tiles
NPAIR = HEADS // 2
VW = HEADS * (DH + 1)  # vext width: per head [v | ones] = 65


def _build():
    nc = bacc.Bacc()
    x_ext = nc.declare_dram_parameter("x", [PB, C, N], F32, isOutput=False)
    wqkvt_ext = nc.declare_dram_parameter("wqkvt", [C, 3 * C], F32, isOutput=False)
    wot_ext = nc.declare_dram_parameter("wot", [C, C], F32, isOutput=False)
    gammat_ext = nc.declare_dram_parameter("gammat", [128, CT], F32, isOutput=False)
    memk_ext = nc.declare_dram_parameter("memk", [128, NPAIR, NMEM], F32, isOutput=False)
    memv_ext = nc.declare_dram_parameter("memv", [128, 2, VW], F32, isOutput=False)
    out_ext = nc.declare_dram_parameter("out", [PB, C, N], F32, isOutput=True)

    with tile.TileContext(nc) as tc:
        with (
            tc.tile_pool(name="const", bufs=1) as const,
            tc.tile_pool(name="wstage", bufs=2) as wstage,
            tc.tile_pool(name="xp", bufs=2) as xp,
            tc.tile_pool(name="data", bufs=1) as data,
            tc.tile_pool(name="atp", bufs=2) as atp,
            tc.tile_pool(name="qp", bufs=2) as qp,
            tc.tile_pool(name="pp", bufs=4) as pp,
            tc.tile_pool(name="pm", bufs=4) as pm,
            tc.tile_pool(name="avs", bufs=4) as avsp,
            tc.tile_pool(name="rp", bufs=4) as rp,
            tc.tile_pool(name="ob", bufs=4) as obp,
            tc.tile_pool(name="qkv_ps", bufs=2, space="PSUM") as qkv_ps,
            tc.tile_pool(name="sim_ps", bufs=2, space="PSUM") as sim_ps,
            tc.tile_pool(name="av_ps", bufs=2, space="PSUM") as av_ps,
        ):
            # ------------ batch-0 x load first (weights stream behind it) -------
            xraws = []
            for bb in range(PB):
                xr = xp.tile([128, CT, N], F32, tag="xraw")
                xraws.append(xr)
            for t in range(CT):
                eng = nc.sync if t < 2 else nc.scalar
                eng.dma_start(out=xraws[0][:, t, :], in_=x_ext[0, t * 128:(t + 1) * 128, :])

            # ---------------- per-core constants ----------------
            wqkv = const.tile([128, CT, 3 * C], BF16, tag="wqkv")
            wo = const.tile([128, CT, C], BF16, tag="wo")
            g1 = const.tile([128, CT], F32, tag="g1")
            g1q = const.tile([128, CT], F32, tag="g1q")
            ones128 = const.tile([128, 128], BF16, tag="ones128")
            ones1 = const.tile([128, 64], F32R, tag="ones1")
            # kT packed per head-pair: rows 0:64 = even head (d), 64:128 = odd
            kTp = const.tile([128, NPAIR, 1028], BF16, tag="kTp")
            vextA = const.tile([128, 8, VW], BF16, tag="vextA")
            vextB = const.tile([128, 8, VW], BF16, tag="vextB")
            vmem = const.tile([128, 2, VW], BF16, tag="vmem")
            vexts = [vextA, vextB]

            gsb = const.tile([128, CT], F32, tag="gsb")
            nc.sync.dma_start(out=gsb, in_=gammat_ext[:, :])
            nc.scalar.activation(out=g1, in_=gsb, func=AF.Copy, bias=1.0)
            nc.scalar.activation(out=g1q, in_=gsb, func=AF.Copy, bias=1.0, scale=1.0)
            nc.scalar.mul(out=g1q, in_=g1q, mul=DH ** -0.5)

            nc.vector.memset(ones128, 1.0)
            nc.vector.memset(ones1.bitcast(F32), 1.0)

            def weight_prep():
                for t in range(CT):
                    ws = wstage.tile([128, 3 * C], F32, tag="ws")
                    nc.sync.dma_start(out=ws, in_=wqkvt_ext[t * 128:(t + 1) * 128, :])
                    nc.vector.tensor_scalar_mul(
                        out=wqkv[:, t, 0:C], in0=ws[:, 0:C], scalar1=g1q[:, t:t + 1])
                    nc.vector.tensor_scalar_mul(
                        out=wqkv[:, t, C:3 * C], in0=ws[:, C:3 * C], scalar1=g1[:, t:t + 1])
                for t in range(CT):
                    ws = wstage.tile([128, 3 * C], F32, tag="ws")
                    nc.sync.dma_start(out=ws[:, 0:C], in_=wot_ext[t * 128:(t + 1) * 128, :])
                    nc.vector.tensor_copy(out=wo[:, t, :], in_=ws[:, 0:C])
                # mem_kv constants
                ws = wstage.tile([128, 3 * C], F32, tag="ws")
                nc.sync.dma_start(out=ws[:, 0:NPAIR * NMEM],
                                  in_=memk_ext[:, :, :].rearrange("p g c -> p (g c)"))
                nc.sync.dma_start(out=ws[:, NPAIR * NMEM:NPAIR * NMEM + 2 * VW],
                                  in_=memv_ext[:, :, :].rearrange("p g c -> p (g c)"))
                nc.vector.tensor_copy(
                    out=kTp[:, :, 1024:1028],
                    in_=ws[:, 0:NPAIR * NMEM].rearrange("p (g c) -> p g c", c=NMEM))
                nc.vector.tensor_copy(
                    out=vmem,
                    in_=ws[:, NPAIR * NMEM:NPAIR * NMEM + 2 * VW].rearrange("p (g c) -> p g c", c=VW))
                for v in vexts:
                    oc = v[:, :, :].rearrange("p j (h c) -> p j h c", c=DH + 1)[:, :, :, DH:DH + 1]
                    nc.gpsimd.memset(oc, 1.0)

            # ---------------- pipeline stages ----------------
            def norm(bb):
                """x -> xn (bf16, per-pixel normalized)."""
                xraw = xraws[bb]
                xsq = data.tile([128, CT, N], BF16, tag="xsq")
                for t in range(CT):
                    nc.vector.tensor_mul(out=xsq[:, t, :], in0=xraw[:, t, :], in1=xraw[:, t, :])
                ss = sim_ps.tile([128, N], F32, tag="sim")
                for h2 in range(2):
                    for t in range(CT):
                        nc.tensor.matmul(ss[:, h2 * 512:(h2 + 1) * 512], ones128,
                                         xsq[:, t, h2 * 512:(h2 + 1) * 512],
                                         start=(t == 0), stop=(t == CT - 1))
                sroot = data.tile([128, N], F32, tag="sroot")
                nc.scalar.activation(out=sroot, in_=ss, func=AF.Sqrt, scale=1.0 / C)
                snorm = data.tile([128, N], F32, tag="snorm")
                nc.vector.reciprocal_approx_fast(out=snorm, in_=sroot)
                xn = data.tile([128, CT, N], BF16, tag="xn" + str(bb))
                for t in range(CT):
                    nc.vector.tensor_mul(out=xn[:, t, :], in0=xraw[:, t, :], in1=snorm)
                return xn

            def qkproj(xn, qT, mcs):
                """o-chunks mcs of the q/k projection; k goes into kTp (paired)."""
                for mc in mcs:
                    for h2 in range(2):
                        ps = qkv_ps.tile([128, 512], F32, tag="q")
                        for t in range(CT):
                            nc.tensor.matmul(ps, wqkv[:, t, mc * 128:(mc + 1) * 128],
                                             xn[:, t, h2 * 512:(h2 + 1) * 512],
                                             start=(t == 0), stop=(t == CT - 1))
                        if mc < 4:
                            nc.vector.tensor_copy(out=qT[:, mc, h2 * 512:(h2 + 1) * 512], in_=ps)
                        else:
                            nc.vector.tensor_copy(
                                out=kTp[:, mc - 4, h2 * 512:(h2 + 1) * 512], in_=ps)

            def vproj(xn, vext, ics):
                for ic in ics:
                    ps = qkv_ps.tile([128, 512], F32, tag="q")
                    for t in range(CT):
                        nc.tensor.matmul(ps, xn[:, t, ic * 128:(ic + 1) * 128],
                                         wqkv[:, t, 2 * C:3 * C],
                                         start=(t == 0), stop=(t == CT - 1))
                    ps_h = ps[:, :].rearrange("p (h c) -> p h c", c=DH)
                    vdst = vext[:, ic, :].rearrange("p (h c) -> p h c", c=DH + 1)[:, :, 0:DH]
                    nc.vector.tensor_copy(out=vdst, in_=ps_h)

            def mem_sims(qT):
                """pmem[g] rows 32*(h%4):+4 = exp(k_mem^T q) for head h=4g+h4."""
                pms = []
                for g in range(2):
                    st = sim_ps.tile([128, N], F32, tag="sim")
                    for h4 in range(4):
                        h = 4 * g + h4
                        p, hh = h // 2, h % 2
                        for h2 in range(2):
                            nc.tensor.matmul(
                                st[32 * h4:32 * h4 + NMEM, h2 * 512:(h2 + 1) * 512],
                                kTp[64 * hh:64 * hh + 64, p, 1024:1028],
                                qT[64 * hh:64 * hh + 64, p, h2 * 512:(h2 + 1) * 512],
                                start=True, stop=True, tile_position=(64 * hh, 32 * h4))
                    pmt = pm.tile([128, N], BF16, tag="pm")
                    nc.scalar.activation(out=pmt, in_=st, func=AF.Exp)
                    pms.append(pmt)
                return pms

            def pair_attn(p, qT, vext, attn, pmem, fill):
                """Attention for heads (2p, 2p+1); fill = deferred work slots."""
                for h2 in range(2):
                    avA = av_ps.tile([65, 512], F32, tag="av")
                    avB = av_ps.tile([65, 512], F32, tag="av")
                    avt = (avA, avB)
                    for jc in range(8):
                        st = sim_ps.tile([128, N], F32, tag="sim")
                        for hh in range(2):
                            nc.tensor.matmul(
                                st[:, hh * 512:(hh + 1) * 512],
                                kTp[64 * hh:64 * hh + 64, p, jc * 128:(jc + 1) * 128],
                                qT[64 * hh:64 * hh + 64, p, h2 * 512:(h2 + 1) * 512],
                                start=True, stop=True)
                        pt = pp.tile([128, N], BF16, tag="p")
                        nc.scalar.activation(out=pt, in_=st, func=AF.Exp)
                        for hh in range(2):
                            h = 2 * p + hh
                            nc.tensor.matmul(
                                avt[hh], vext[:, jc, h * (DH + 1):(h + 1) * (DH + 1)],
                                pt[:, hh * 512:(hh + 1) * 512],
                                start=(jc == 0), stop=False)
                        if fill and jc in (3, 7):
                            fill.pop(0)()
                    # mem_kv contribution
                    for hh in range(2):
                        h = 2 * p + hh
                        g, r0 = h // 4, 32 * (h % 4)
                        nc.tensor.matmul(
                            avt[hh],
                            vmem[r0:r0 + NMEM, g, (h % 4) * (DH + 1):(h % 4 + 1) * (DH + 1)],
                            pmem[g][r0:r0 + NMEM, h2 * 512:(h2 + 1) * 512],
                            start=False, stop=True, tile_position=(r0, 0))
                    # evacuate + normalize: attn = av[0:64] / av[64]
                    for hh in range(2):
                        avb = avsp.tile([65, 512], F32R, tag="avs")
                        with tc.high_priority(offset=64):
                            nc.vector.tensor_copy(out=avb, in_=avt[hh])
                        bc = qkv_ps.tile([64, 512], F32, tag="q")
                        nc.tensor.matmul(bc, ones1[64:65, :], avb[64:65, :], start=True, stop=True)
                        rcp = rp.tile([64, 512], F32, tag="rcp")
                        nc.vector.reciprocal_approx_fast(out=rcp, in_=bc)
                        nc.vector.tensor_mul(
                            out=attn[64 * hh:64 * hh + 64, p, h2 * 512:(h2 + 1) * 512],
                            in0=avb[0:64, :].bitcast(F32), in1=rcp)

            def proj(attn, bb, mcs=None, h2s=(0, 1)):
                for mc in (range(CT) if mcs is None else mcs):
                    for h2 in h2s:
                        ps = qkv_ps.tile([128, 512], F32, tag="q")
                        for t in range(CT):
                            nc.tensor.matmul(ps, wo[:, t, mc * 128:(mc + 1) * 128],
                                             attn[:, t, h2 * 512:(h2 + 1) * 512],
                                             start=(t == 0), stop=(t == CT - 1))
                        ob = obp.tile([128, 512], F32, tag="ob")
                        nc.vector.tensor_copy(out=ob, in_=ps)
                        nc.sync.dma_start(
                            out=out_ext[bb, mc * 128:(mc + 1) * 128, h2 * 512:(h2 + 1) * 512],
                            in_=ob)

            # ---------------- interleaved schedule ----------------
            xn0 = norm(0)
            weight_prep()
            for t in range(CT):
                nc.sync.dma_start(out=xraws[1][:, t, :], in_=x_ext[1, t * 128:(t + 1) * 128, :])
            qT0 = qp.tile([128, CT, N], BF16, tag="qT")
            qkproj(xn0, qT0, range(8))
            vproj(xn0, vexts[0], range(8))
            xn1 = norm(1)

            pmem0 = mem_sims(qT0)
            qT1 = qp.tile([128, CT, N], BF16, tag="qT")
            attn0 = atp.tile([128, CT, N], BF16, tag="attn")
            # batch-1 q/v/k chunks fill the exp-bound bubbles of batch-0
            # attention. k chunk for pair i (qkproj [4+i]) overwrites kTp
            # pair i, so it may only be issued once batch-0's sweep (i, 1)
            # has fully read it — i.e. from sweep i+1 onward. Pair 3's k
            # is issued after the whole batch-0 loop.
            fill0 = [
                lambda: qkproj(xn1, qT1, [0]),                # (0,0) jc3
                lambda: vproj(xn1, vexts[1], [0, 1]),         # (0,0) jc7
                lambda: qkproj(xn1, qT1, [1]),                # (0,1) jc3
                lambda: vproj(xn1, vexts[1], [2, 3]),         # (0,1) jc7
                lambda: qkproj(xn1, qT1, [2]),                # (1,0) jc3
                lambda: qkproj(xn1, qT1, [4]),                # (1,0) jc7: k pair 0
                lambda: qkproj(xn1, qT1, [3]),                # (1,1) jc3
                lambda: vproj(xn1, vexts[1], [4, 5]),         # (1,1) jc7
                lambda: qkproj(xn1, qT1, [5]),                # (2,0) jc3: k pair 1
                lambda: vproj(xn1, vexts[1], [6, 7]),         # (2,0) jc7
                lambda: None,                                 # (2,1) jc3
                lambda: None,                                 # (2,1) jc7
                lambda: qkproj(xn1, qT1, [6]),                # (3,0) jc3: k pair 2
                lambda: None,                                 # (3,0) jc7
                lambda: None,                                 # (3,1) jc3
                lambda: None,                                 # (3,1) jc7
            ]
            for p in range(NPAIR):
                pair_attn(p, qT0, vexts[0], attn0, pmem0, fill0)

            qkproj(xn1, qT1, [7])                             # k pair 3
            pmem1 = mem_sims(qT1)
            attn1 = atp.tile([128, CT, N], BF16, tag="attn")
            # batch-0 out-projection fills batch-1 attention bubbles
            fill1 = []
            for mc in range(CT):
                for h2 in range(2):
                    fill1.append(lambda mc=mc, h2=h2: proj(attn0, 0, [mc], (h2,)))
                fill1.append(lambda: None)
                fill1.append(lambda: None)
            for p in range(NPAIR):
                pair_attn(p, qT1, vexts[1], attn1, pmem1, fill1)
            proj(attn1, 1)
    nc.compile()
    return nc


_NC_CACHE = []


def kernel(x, gamma, mem_kv, w_qkv, w_out, _trace=False):
    x = np.asarray(x, dtype=np.float32)
    gamma = np.asarray(gamma, dtype=np.float32)
    mem_kv = np.asarray(mem_kv, dtype=np.float32)
    w_qkv = np.asarray(w_qkv, dtype=np.float32)
    w_out = np.asarray(w_out, dtype=np.float32)

    b, c, hh, ww = x.shape
    n = hh * ww
    xs = x.reshape(b, c, n)

    wqkvt = np.ascontiguousarray(w_qkv.T)          # [c, 3c]
    wot = np.ascontiguousarray(w_out.T)            # [c, c]
    gammat = np.ascontiguousarray(gamma.reshape(CT, 128).T)  # [128, CT]

    memk = np.zeros((128, NPAIR, NMEM), np.float32)
    memv = np.zeros((128, 2, VW), np.float32)
    for h in range(HEADS):
        p, hh_ = h // 2, h % 2
        memk[64 * hh_:64 * hh_ + DH, p, 0:NMEM] = mem_kv[0, h].T  # [dh, nmem]
        g, r1, c0 = h // 4, 32 * (h % 4), (h % 4) * (DH + 1)
        memv[r1:r1 + NMEM, g, c0:c0 + DH] = mem_kv[1, h]
        memv[r1:r1 + NMEM, g, c0 + DH] = 1.0

    if not _NC_CACHE:
        _NC_CACHE.append(_build())
    nc = _NC_CACHE[0]

    in_maps = []
    for core in range(NCORES):
        in_maps.append({
            "x": np.ascontiguousarray(xs[core * PB:(core + 1) * PB]),
            "wqkvt": wqkvt,
            "wot": wot,
            "gammat": gammat,
            "memk": memk,
            "memv": memv,
        })
    res = run_bass_kernel_spmd(nc, in_maps, core_ids=list(range(NCORES)), trace=_trace)
    out = np.concatenate([res.results[core]["out"] for core in range(NCORES)], axis=0)
    kernel.last_result = res
    return out.reshape(b, c, hh, ww)


# revision 8
# speedup vs baseline: 1.0489x; 1.0489x over previous
"""Trainium2 Bass kernel for nn_Attention_7945689497706.

v2: pair-processed attention with K=64 row-tiled sim matmuls (PE row
tiling runs both heads' 64-contraction sims concurrently instead of
zero-padding to 128), sim tile [128, 1024] = [h0 i-half | h1 i-half]
so exp stays at full [128,1024] ACT size.
"""

import numpy as np

import concourse.bass as bass
import concourse.mybir as mybir
import concourse.tile as tile
from concourse import bacc
from concourse.bass_utils import run_bass_kernel_spmd

F32 = mybir.dt.float32
F32R = mybir.dt.float32r
BF16 = mybir.dt.bfloat16
AF = mybir.ActivationFunctionType

NCORES = 8
B = 16
C = 512
N = 1024          # pixels = 32*32
HEADS = 8
DH = 64
NMEM = 4
PB = B // NCORES  # batch elements per core
CT = C // 128     # channel partition-tiles
NPAIR = HEADS // 2
VW = HEADS * (DH + 1)  # vext width: per head [v | ones] = 65


def _build():
    nc = bacc.Bacc()
    x_ext = nc.declare_dram_parameter("x", [PB, C, N], F32, isOutput=False)
    wqkvt_ext = nc.declare_dram_parameter("wqkvt", [C, 3 * C], F32, isOutput=False)
    wot_ext = nc.declare_dram_parameter("wot", [C, C], F32, isOutput=False)
    gammat_ext = nc.declare_dram_parameter("gammat", [128, CT], F32, isOutput=False)
    memk_ext = nc.declare_dram_parameter("memk", [128, NPAIR, NMEM], F32, isOutput=False)
    memv_ext = nc.declare_dram_parameter("memv", [128, 2, VW], F32, isOutput=False)
    out_ext = nc.declare_dram_parameter("out", [PB, C, N], F32, isOutput=True)

    with tile.TileContext(nc) as tc:
        with (
            tc.tile_pool(name="const", bufs=1) as const,
            tc.tile_pool(name="wstage", bufs=2) as wstage,
            tc.tile_pool(name="xp", bufs=2) as xp,
            tc.tile_pool(name="data", bufs=1) as data,
            tc.tile_pool(name="atp", bufs=2) as atp,
            tc.tile_pool(name="qp", bufs=2) as qp,
            tc.tile_pool(name="pp", bufs=4) as pp,
            tc.tile_pool(name="pm", bufs=4) as pm,
            tc.tile_pool(name="avs", bufs=4) as avsp,
            tc.tile_pool(name="rp", bufs=4) as rp,
            tc.tile_pool(name="ob", bufs=4) as obp,
            tc.tile_pool(name="qkv_ps", bufs=2, space="PSUM") as qkv_ps,
            tc.tile_pool(name="sim_ps", bufs=2, space="PSUM") as sim_ps,
            tc.tile_pool(name="av_ps", bufs=2, space="PSUM") as av_ps,
        ):
            # ------------ batch-0 x load first (weights stream behind it) -------
            xraws = []
            for bb in range(PB):
                xr = xp.tile([128, CT, N], F32, tag="xraw")
                xraws.append(xr)
            for t in range(CT):
                eng = nc.sync if t < 2 else nc.scalar
                eng.dma_start(out=xraws[0][:, t, :], in_=x_ext[0, t * 128:(t + 1) * 128, :])

            # ---------------- per-core constants ----------------
            wqkv = const.tile([128, CT, 3 * C], BF16, tag="wqkv")
            wo = const.tile([128, CT, C], BF16, tag="wo")
            g1 = const.tile([128, CT], F32, tag="g1")
            g1q = const.tile([128, CT], F32, tag="g1q")
            ones128 = const.tile([128, 128], BF16, tag="ones128")
            ones1 = const.tile([128, 64], F32R, tag="ones1")
            # kT packed per head-pair: rows 0:64 = even head (d), 64:128 = odd
            kTp = const.tile([128, NPAIR, 1028], BF16, tag="kTp")
            vextA = const.tile([128, 8, VW], BF16, tag="vextA")
            vextB = const.tile([128, 8, VW], BF16, tag="vextB")
            vmem = const.tile([128, 2, VW], BF16, tag="vmem")
            vexts = [vextA, vextB]

            gsb = const.tile([128, CT], F32, tag="gsb")
            nc.sync.dma_start(out=gsb, in_=gammat_ext[:, :])
            nc.scalar.activation(out=g1, in_=gsb, func=AF.Copy, bias=1.0)
            nc.scalar.activation(out=g1q, in_=gsb, func=AF.Copy, bias=1.0, scale=1.0)
            nc.scalar.mul(out=g1q, in_=g1q, mul=DH ** -0.5)

            nc.vector.memset(ones128, 1.0)
            nc.vector.memset(ones1.bitcast(F32), 1.0)

            def weight_prep():
                for t in range(CT):
                    ws = wstage.tile([128, 3 * C], F32, tag="ws")
                    nc.sync.dma_start(out=ws, in_=wqkvt_ext[t * 128:(t + 1) * 128, :])
                    nc.vector.tensor_scalar_mul(
                        out=wqkv[:, t, 0:C], in0=ws[:, 0:C], scalar1=g1q[:, t:t + 1])
                    nc.vector.tensor_scalar_mul(
                        out=wqkv[:, t, C:3 * C], in0=ws[:, C:3 * C], scalar1=g1[:, t:t + 1])
                for t in range(CT):
                    ws = wstage.tile([128, 3 * C], F32, tag="ws")
                    nc.sync.dma_start(out=ws[:, 0:C], in_=wot_ext[t * 128:(t + 1) * 128, :])
                    nc.vector.tensor_copy(out=wo[:, t, :], in_=ws[:, 0:C])
                # mem_kv constants
                ws = wstage.tile([128, 3 * C], F32, tag="ws")
                nc.sync.dma_start(out=ws[:, 0:NPAIR * NMEM],
                                  in_=memk_ext[:, :, :].rearrange("p g c -> p (g c)"))
                nc.sync.dma_start(out=ws[:, NPAIR * NMEM:NPAIR * NMEM + 2 * VW],
                                  in_=memv_ext[:, :, :].rearrange("p g c -> p (g c)"))
                nc.vector.tensor_copy(
                    out=kTp[:, :, 1024:1028],
                    in_=ws[:, 0:NPAIR * NMEM].rearrange("p (g c) -> p g c", c=NMEM))
                nc.vector.tensor_copy(
                    out=vmem,
                    in_=ws[:, NPAIR * NMEM:NPAIR * NMEM + 2 * VW].rearrange("p (g c) -> p g c", c=VW))
                for v in vexts:
                    oc = v[:, :, :].rearrange("p j (h c) -> p j h c", c=DH + 1)[:, :, :, DH:DH + 1]
                    nc.gpsimd.memset(oc, 1.0)

            # ---------------- pipeline stages ----------------
            def norm(bb):
                """x -> xn (bf16, per-pixel normalized)."""
                xraw = xraws[bb]
                xsq = data.tile([128, CT, N], BF16, tag="xsq")
                for t in range(CT):
                    nc.vector.tensor_mul(out=xsq[:, t, :], in0=xraw[:, t, :], in1=xraw[:, t, :])
                ss = sim_ps.tile([128, N], F32, tag="sim")
                for h2 in range(2):
                    for t in range(CT):
                        nc.tensor.matmul(ss[:, h2 * 512:(h2 + 1) * 512], ones128,
                                         xsq[:, t, h2 * 512:(h2 + 1) * 512],
                                         start=(t == 0), stop=(t == CT - 1))
                sroot = data.tile([128, N], F32, tag="sroot")
                nc.scalar.activation(out=sroot, in_=ss, func=AF.Sqrt, scale=1.0 / C)
                snorm = data.tile([128, N], F32, tag="snorm")
                nc.vector.reciprocal_approx_fast(out=snorm, in_=sroot)
                xn = data.tile([128, CT, N], BF16, tag="xn" + str(bb))
                for t in range(CT):
                    nc.vector.tensor_mul(out=xn[:, t, :], in0=xraw[:, t, :], in1=snorm)
                return xn

            def qkproj(xn, qT, mcs):
                """o-chunks mcs of the q/k projection; k goes into kTp (paired)."""
                for mc in mcs:
                    for h2 in range(2):
                        ps = qkv_ps.tile([128, 512], F32, tag="q")
                        for t in range(CT):
                            nc.tensor.matmul(ps, wqkv[:, t, mc * 128:(mc + 1) * 128],
                                             xn[:, t, h2 * 512:(h2 + 1) * 512],
                                             start=(t == 0), stop=(t == CT - 1))
                        if mc < 4:
                            nc.vector.tensor_copy(out=qT[:, mc, h2 * 512:(h2 + 1) * 512], in_=ps)
                        else:
                            nc.vector.tensor_copy(
                                out=kTp[:, mc - 4, h2 * 512:(h2 + 1) * 512], in_=ps)

            def qkproj_units(xn, qT, mc):
                """qkproj chunk split into 2-MM fill units (4 units per chunk)."""
                state = {}
                units = []
                for h2 in range(2):
                    def u1(h2=h2):
                        ps = qkv_ps.tile([128, 512], F32, tag="q")
                        state[h2] = ps
                        for t in (0, 1):
                            nc.tensor.matmul(ps, wqkv[:, t, mc * 128:(mc + 1) * 128],
                                             xn[:, t, h2 * 512:(h2 + 1) * 512],
                                             start=(t == 0), stop=False)

                    def u2(h2=h2):
                        ps = state[h2]
                        for t in (2, 3):
                            nc.tensor.matmul(ps, wqkv[:, t, mc * 128:(mc + 1) * 128],
                                             xn[:, t, h2 * 512:(h2 + 1) * 512],
                                             start=False, stop=(t == CT - 1))
                        if mc < 4:
                            nc.vector.tensor_copy(out=qT[:, mc, h2 * 512:(h2 + 1) * 512], in_=ps)
                        else:
                            nc.vector.tensor_copy(
                                out=kTp[:, mc - 4, h2 * 512:(h2 + 1) * 512], in_=ps)
                    units.append(u1)
                    units.append(u2)
                return units

            def vproj(xn, vext, ics):
                for ic in ics:
                    ps = qkv_ps.tile([128, 512], F32, tag="q")
                    for t in range(CT):
                        nc.tensor.matmul(ps, xn[:, t, ic * 128:(ic + 1) * 128],
                                         wqkv[:, t, 2 * C:3 * C],
                                         start=(t == 0), stop=(t == CT - 1))
                    ps_h = ps[:, :].rearrange("p (h c) -> p h c", c=DH)
                    vdst = vext[:, ic, :].rearrange("p (h c) -> p h c", c=DH + 1)[:, :, 0:DH]
                    nc.vector.tensor_copy(out=vdst, in_=ps_h)

            def vproj_units(xn, vext, ic):
                """vproj chunk split into 2-MM fill units (2 units per chunk)."""
                state = {}

                def u1():
                    ps = qkv_ps.tile([128, 512], F32, tag="q")
                    state[0] = ps
                    for t in (0, 1):
                        nc.tensor.matmul(ps, xn[:, t, ic * 128:(ic + 1) * 128],
                                         wqkv[:, t, 2 * C:3 * C],
                                         start=(t == 0), stop=False)

                def u2():
                    ps = state[0]
                    for t in (2, 3):
                        nc.tensor.matmul(ps, xn[:, t, ic * 128:(ic + 1) * 128],
                                         wqkv[:, t, 2 * C:3 * C],
                                         start=False, stop=(t == CT - 1))
                    ps_h = ps[:, :].rearrange("p (h c) -> p h c", c=DH)
                    vdst = vext[:, ic, :].rearrange("p (h c) -> p h c", c=DH + 1)[:, :, 0:DH]
                    nc.vector.tensor_copy(out=vdst, in_=ps_h)
                return [u1, u2]

            def mem_sims(qT):
                """pmem[g] rows 32*(h%4):+4 = exp(k_mem^T q) for head h=4g+h4."""
                pms = []
                for g in range(2):
                    st = sim_ps.tile([128, N], F32, tag="sim")
                    for h4 in range(4):
                        h = 4 * g + h4
                        p, hh = h // 2, h % 2
                        for h2 in range(2):
                            nc.tensor.matmul(
                                st[32 * h4:32 * h4 + NMEM, h2 * 512:(h2 + 1) * 512],
                                kTp[64 * hh:64 * hh + 64, p, 1024:1028],
                                qT[64 * hh:64 * hh + 64, p, h2 * 512:(h2 + 1) * 512],
                                start=True, stop=True, tile_position=(64 * hh, 32 * h4))
                    pmt = pm.tile([128, N], BF16, tag="pm")
                    nc.scalar.activation(out=pmt, in_=st, func=AF.Exp)
                    pms.append(pmt)
                return pms

            def pair_attn(p, qT, vext, attn, pmem, fill):
                """Attention for heads (2p, 2p+1); fill = deferred work units.

                av matmuls lag the sims by 2 rounds so they never sit at the
                head of the in-order PE queue waiting on an exp: av(jc) only
                issues once exp(jc) finished ~2 rounds ago.
                """
                for h2 in range(2):
                    avA = av_ps.tile([65, 512], F32, tag="av")
                    avB = av_ps.tile([65, 512], F32, tag="av")
                    avt = (avA, avB)
                    pts = [None] * 8

                    def av_mms(jc):
                        for hh in range(2):
                            h = 2 * p + hh
                            nc.tensor.matmul(
                                avt[hh], vext[:, jc, h * (DH + 1):(h + 1) * (DH + 1)],
                                pts[jc][:, hh * 512:(hh + 1) * 512],
                                start=(jc == 0), stop=False)

                    for jc in range(8):
                        st = sim_ps.tile([128, N], F32, tag="sim")
                        for hh in range(2):
                            nc.tensor.matmul(
                                st[:, hh * 512:(hh + 1) * 512],
                                kTp[64 * hh:64 * hh + 64, p, jc * 128:(jc + 1) * 128],
                                qT[64 * hh:64 * hh + 64, p, h2 * 512:(h2 + 1) * 512],
                                start=True, stop=True)
                        if jc >= 2:
                            av_mms(jc - 2)
                        if fill:
                            fill.pop(0)()
                        pt = pp.tile([128, N], BF16, tag="p")
                        nc.scalar.activation(out=pt, in_=st, func=AF.Exp)
                        pts[jc] = pt
                    av_mms(6)
                    av_mms(7)
                    # mem_kv contribution
                    for hh in range(2):
                        h = 2 * p + hh
                        g, r0 = h // 4, 32 * (h % 4)
                        nc.tensor.matmul(
                            avt[hh],
                            vmem[r0:r0 + NMEM, g, (h % 4) * (DH + 1):(h % 4 + 1) * (DH + 1)],
                            pmem[g][r0:r0 + NMEM, h2 * 512:(h2 + 1) * 512],
                            start=False, stop=True, tile_position=(r0, 0))
                    # evacuate + normalize: attn = av[0:64] / av[64]
                    for hh in range(2):
                        avb = avsp.tile([65, 512], F32R, tag="avs")
                        with tc.high_priority(offset=64):
                            nc.vector.tensor_copy(out=avb, in_=avt[hh])
                        bc = qkv_ps.tile([64, 512], F32, tag="q")
                        nc.tensor.matmul(bc, ones1[64:65, :], avb[64:65, :], start=True, stop=True)
                        rcp = rp.tile([64, 512], F32, tag="rcp")
                        nc.vector.reciprocal_approx_fast(out=rcp, in_=bc)
                        nc.vector.tensor_mul(
                            out=attn[64 * hh:64 * hh + 64, p, h2 * 512:(h2 + 1) * 512],
                            in0=avb[0:64, :].bitcast(F32), in1=rcp)

            def proj(attn, bb, mcs=None, h2s=(0, 1)):
                for mc in (range(CT) if mcs is None else mcs):
                    for h2 in h2s:
                        ps = qkv_ps.tile([128, 512], F32, tag="q")
                        for t in range(CT):
                            nc.tensor.matmul(ps, wo[:, t, mc * 128:(mc + 1) * 128],
                                             attn[:, t, h2 * 512:(h2 + 1) * 512],
                                             start=(t == 0), stop=(t == CT - 1))
                        ob = obp.tile([128, 512], F32, tag="ob")
                        nc.vector.tensor_copy(out=ob, in_=ps)
                        nc.sync.dma_start(
                            out=out_ext[bb, mc * 128:(mc + 1) * 128, h2 * 512:(h2 + 1) * 512],
                            in_=ob)

            # ---------------- interleaved schedule ----------------
            xn0 = norm(0)
            weight_prep()
            for t in range(CT):
                nc.sync.dma_start(out=xraws[1][:, t, :], in_=x_ext[1, t * 128:(t + 1) * 128, :])
            qT0 = qp.tile([128, CT, N], BF16, tag="qT")
            qkproj(xn0, qT0, range(8))
            vproj(xn0, vexts[0], range(8))
            xn1 = norm(1)

            pmem0 = mem_sims(qT0)
            qT1 = qp.tile([128, CT, N], BF16, tag="qT")
            attn0 = atp.tile([128, CT, N], BF16, tag="attn")
            # batch-1 q/v/k chunks fill the exp-bound bubbles of batch-0
            # attention, as fine-grained 2-MM units (one pop per round).
            # k chunk for pair i (qkproj [4+i]) overwrites kTp pair i, so
            # its units may only be issued from sweep index 2i+2 onward
            # (sweep index = 2p+h2); pair 3's k goes after the whole loop.
            nop = lambda: None
            sweeps0 = [
                qkproj_units(xn1, qT1, 0) + vproj_units(xn1, vexts[1], 0) + [nop, nop],
                qkproj_units(xn1, qT1, 1) + vproj_units(xn1, vexts[1], 1) + [nop, nop],
                qkproj_units(xn1, qT1, 4) + vproj_units(xn1, vexts[1], 2) + [nop, nop],
                qkproj_units(xn1, qT1, 2) + vproj_units(xn1, vexts[1], 3) + [nop, nop],
                qkproj_units(xn1, qT1, 5) + vproj_units(xn1, vexts[1], 4) + [nop, nop],
                qkproj_units(xn1, qT1, 3) + vproj_units(xn1, vexts[1], 5) + [nop, nop],
                qkproj_units(xn1, qT1, 6) + vproj_units(xn1, vexts[1], 6) + [nop, nop],
                vproj_units(xn1, vexts[1], 7) + [nop] * 6,
            ]
            fill0 = [u for sweep in sweeps0 for u in sweep]
            for p in range(NPAIR):
                pair_attn(p, qT0, vexts[0], attn0, pmem0, fill0)

            qkproj(xn1, qT1, [7])                             # k pair 3
            pmem1 = mem_sims(qT1)
            attn1 = atp.tile([128, CT, N], BF16, tag="attn")
            # batch-0 out-projection fills batch-1 attention bubbles
            fill1 = []
            for mc in range(CT):
                for h2 in range(2):
                    fill1.append(lambda mc=mc, h2=h2: proj(attn0, 0, [mc], (h2,)))
                    fill1.append(nop)
                    fill1.append(nop)
                    fill1.append(nop)
            for p in range(NPAIR):
                pair_attn(p, qT1, vexts[1], attn1, pmem1, fill1)
            proj(attn1, 1)
    nc.compile()
    return nc


_NC_CACHE = []


def kernel(x, gamma, mem_kv, w_qkv, w_out, _trace=False):
    x = np.asarray(x, dtype=np.float32)
    gamma = np.asarray(gamma, dtype=np.float32)
    mem_kv = np.asarray(mem_kv, dtype=np.float32)
    w_qkv = np.asarray(w_qkv, dtype=np.float32)
    w_out = np.asarray(w_out, dtype=np.float32)

    b, c, hh, ww = x.shape
    n = hh * ww
    xs = x.reshape(b, c, n)

    wqkvt = np.ascontiguousarray(w_qkv.T)          # [c, 3c]
    wot = np.ascontiguousarray(w_out.T)            # [c, c]
    gammat = np.ascontiguousarray(gamma.reshape(CT, 128).T)  # [128, CT]

    memk = np.zeros((128, NPAIR, NMEM), np.float32)
    memv = np.zeros((128, 2, VW), np.float32)
    for h in range(HEADS):
        p, hh_ = h // 2, h % 2
        memk[64 * hh_:64 * hh_ + DH, p, 0:NMEM] = mem_kv[0, h].T  # [dh, nmem]
        g, r1, c0 = h // 4, 32 * (h % 4), (h % 4) * (DH + 1)
        memv[r1:r1 + NMEM, g, c0:c0 + DH] = mem_kv[1, h]
        memv[r1:r1 + NMEM, g, c0 + DH] = 1.0

    if not _NC_CACHE:
        _NC_CACHE.append(_build())
    nc = _NC_CACHE[0]

    in_maps = []
    for core in range(NCORES):
        in_maps.append({
            "x": np.ascontiguousarray(xs[core * PB:(core + 1) * PB]),
            "wqkvt": wqkvt,
            "wot": wot,
            "gammat": gammat,
            "memk": memk,
            "memv": memv,
        })
    res = run_bass_kernel_spmd(nc, in_maps, core_ids=list(range(NCORES)), trace=_trace)
    out = np.concatenate([res.results[core]["out"] for core in range(NCORES)], axis=0)
    kernel.last_result = res
    return out.reshape(b, c, hh, ww)
